# revision 8
# baseline (speedup 1.0000x reference)
"""IsometricLoss on 8 Trainium2 NeuronCores (data-parallel over N).

loss = sum(r * max(||x||^2 + ||mu||^2 - 2 x.mu, 0)) / N

For random-normal X/mus the squared distances are ~2*D >> fp32 noise, so the
max(.,0) clamp never binds and the loss decomposes exactly:

  loss*N = sum_{m,d} S1[m,d] * (-2 mus[m,d])     (cross term,  S1 = r^T X)
         + sum_{m,d} S2[m,d]                     (||x||^2 term, S2 = r^T X.^2)
         + sum_m    rc[m] * mu2[m]               (||mu||^2 term, rc = r^T 1)

Each core streams its N/8 shard of X and r exactly once (memory-bound).
Rows are packed 16-per-partition so each DMA descriptor moves 8 KiB.
Per 128-row group a single bf16 PE matmul accumulates

    psum[128, 257] += r_grp^T @ [x_bf16 | x.^2_bf16 | ones]

into one PSUM bank (fp32 accumulate).  bf16 inputs are safe: the loss is
linear in r and the round-to-nearest input errors (~2^-9 relative, zero
mean) cancel statistically over the 16M-product sums (~1e-6 net).

Pipelining: X rides the Sync HWDGE ring while r rides the Scalar HWDGE ring
(parallel first loads), mus goes via GpSimd SWDGE, and the iteration sizes
ramp 4,8,16,...,16,4 row-groups so the first matmul starts ~4us in and the
drain tail is short.  ScalarE squares X into the comb tile, VectorE casts
X and r to bf16, and a tiny tail contracts psum against
Mext = [-2*mus | 1 | mu2] to a per-core scalar.  Host sums the 8 partials.
"""

from contextlib import ExitStack

import numpy as np

import concourse.bass as bass
import concourse.tile as tile
from concourse import bacc, mybir
from concourse.bass_utils import run_bass_kernel_spmd

N, M, D = 131072, 128, 128
NCORES = 8
NSHARD = N // NCORES          # 16384 rows per core
P = 128                       # SBUF partitions
GMAX = 16                     # row-groups per iteration at steady state
# rows per iteration = P * G_i; ramp in and out for pipeline fill/drain
GSCHED = [4, 8] + [GMAX] * 6 + [8, 8, 4]
assert sum(GSCHED) == NSHARD // P
W = 2 * D + 1                 # moving cols per group: [x(128) | x^2(128) | 1]

F32 = mybir.dt.float32
BF16 = mybir.dt.bfloat16
_cache: dict[str, object] = {}


def _build():
    nc = bacc.Bacc(
        "TRN2",
        target_bir_lowering=False,
        debug=False,
        num_devices=NCORES,
    )

    x_d = nc.dram_tensor("X", [NSHARD, D], F32, kind="ExternalInput").ap()
    r_d = nc.dram_tensor("r", [NSHARD, M], F32, kind="ExternalInput").ap()
    mus_d = nc.dram_tensor("mus", [M, D], F32, kind="ExternalInput").ap()
    out_d = nc.dram_tensor("out", [1, 1], F32, kind="ExternalOutput").ap()

    NGRP = sum(GSCHED)

    with tile.TileContext(nc) as tc, ExitStack() as ctx:
        singles = ctx.enter_context(tc.tile_pool(name="singles", bufs=1))
        big = ctx.enter_context(tc.tile_pool(name="big", bufs=3))
        psum_pool = ctx.enter_context(tc.tile_pool(name="psum", bufs=1, space="PSUM"))

        mus_t = singles.tile([M, D], F32)
        nc.gpsimd.dma_start(mus_t[:], mus_d[:])

        ones_col = singles.tile([P, 1], F32)
        nc.vector.memset(ones_col[:], 1.0)

        psum_acc = psum_pool.tile([M, W], F32)

        row0 = 0
        grp0 = 0
        for it, G in enumerate(GSCHED):
            rows = P * G
            # row = row0 + p*G + g  ->  AP dims [p, g, d]
            x_src = x_d[row0 : row0 + rows, :].rearrange("(p g) d -> p g d", g=G)
            r_src = r_d[row0 : row0 + rows, :].rearrange("(p g) m -> p g m", g=G)

            x_f32 = big.tile([P, G, D], F32, tag="x_f32")
            r_f32 = big.tile([P, G, M], F32, tag="r_f32")
            comb = big.tile([P, G, W], BF16, tag="comb")
            r_bf = big.tile([P, G, M], BF16, tag="r_bf")

            if it < 2:
                # Cold-start: the sync HWDGE ring serializes its first few
                # DMAs (~2.5us each); spread the ramp iterations across the
                # otherwise-idle gpsimd/scalar rings so x, r and mus load in
                # parallel.
                nc.gpsimd.dma_start(x_f32[:], x_src)
                nc.scalar.dma_start(r_f32[:], r_src)
            else:
                nc.sync.dma_start(x_f32[:], x_src)
                nc.sync.dma_start(r_f32[:], r_src)

            nc.vector.tensor_copy(comb[:, :, 0:D], x_f32[:])
            nc.scalar.activation(
                comb[:, :, D : 2 * D], x_f32[:], mybir.ActivationFunctionType.Square
            )
            nc.vector.tensor_copy(r_bf[:], r_f32[:])
            nc.vector.memset(comb[:, :, 2 * D : 2 * D + 1], 1.0)

            for g in range(G):
                idx = grp0 + g
                nc.tensor.matmul(
                    psum_acc[:],
                    r_bf[:, g, :],
                    comb[:, g, :],
                    start=(idx == 0),
                    stop=(idx == NGRP - 1),
                )
            row0 += rows
            grp0 += G

            if it == 2:
                # Mext = [-2*mus | 1 | mu2]; built mid-kernel so it neither
                # delays the first DMAs nor sits on the critical tail path.
                mext = singles.tile([M, W], F32)
                nc.scalar.mul(mext[:, 0:D], mus_t[:], -2.0)
                nc.vector.memset(mext[:, D : 2 * D], 1.0)
                mus_sq = singles.tile([M, D], F32)
                nc.vector.tensor_mul(mus_sq[:], mus_t[:], mus_t[:])
                nc.vector.reduce_sum(
                    mext[:, 2 * D : 2 * D + 1], mus_sq[:], axis=mybir.AxisListType.X
                )

        # Tail: scalar partial = sum(psum_acc * Mext)
        prod = singles.tile([M, W], F32)
        nc.vector.tensor_mul(prod[:], psum_acc[:], mext[:])
        u = singles.tile([M, 1], F32)
        nc.vector.reduce_sum(u[:], prod[:], axis=mybir.AxisListType.X)
        psum_s = psum_pool.tile([1, 1], F32)
        nc.tensor.matmul(psum_s[:], u[:], ones_col[:], start=True, stop=True)
        res = singles.tile([1, 1], F32)
        nc.vector.tensor_copy(res[:], psum_s[:])
        nc.sync.dma_start(out_d[:], res[:])

    nc.compile()
    return nc


def _get_nc():
    if "nc" not in _cache:
        _cache["nc"] = _build()
    return _cache["nc"]


def _run(X, r, mus, **spmd_kwargs):
    X = np.ascontiguousarray(np.asarray(X, dtype=np.float32))
    r = np.ascontiguousarray(np.asarray(r, dtype=np.float32))
    mus = np.ascontiguousarray(np.asarray(mus, dtype=np.float32))
    assert X.shape == (N, D) and r.shape == (N, M) and mus.shape == (M, D)

    nc = _get_nc()
    in_maps = [
        {
            "X": X[c * NSHARD : (c + 1) * NSHARD],
            "r": r[c * NSHARD : (c + 1) * NSHARD],
            "mus": mus,
        }
        for c in range(NCORES)
    ]
    return run_bass_kernel_spmd(nc, in_maps, core_ids=list(range(NCORES)), **spmd_kwargs)


def kernel(X, r, mus):
    out = _run(X, r, mus)
    total = sum(float(out.results[c]["out"][0, 0]) for c in range(NCORES))
    return np.float32(total / N)


# revision 10
# speedup vs baseline: 1.1057x; 1.1057x over previous
"""IsometricLoss on 8 Trainium2 NeuronCores (data-parallel over N).

loss = sum(r * max(||x||^2 + ||mu||^2 - 2 x.mu, 0)) / N

For random-normal X/mus the squared distances are ~2*D >> fp32 noise, so the
max(.,0) clamp never binds and the loss decomposes exactly:

  loss*N = sum_{m,d} S1[m,d] * (-2 mus[m,d])     (cross term,  S1 = r^T X)
         + sum_{m,d} S2[m,d]                     (||x||^2 term, S2 = r^T X.^2)
         + sum_m    rc[m] * mu2[m]               (||mu||^2 term, rc = r^T 1)

Each core streams its N/8 shard of X and r exactly once (memory-bound).
Rows are packed 16-per-partition so each DMA descriptor moves 8 KiB.
Per 128-row group a single bf16 PE matmul accumulates

    psum[128, 257] += r_grp^T @ [x_bf16 | x.^2_bf16 | ones]

into one PSUM bank (fp32 accumulate).  bf16 inputs are safe: the loss is
linear in r and the round-to-nearest input errors (~2^-9 relative, zero
mean) cancel statistically over the 16M-product sums (~1e-6 net).

Pipelining: X rides the Sync HWDGE ring while r rides the Scalar HWDGE ring
(parallel first loads), mus goes via GpSimd SWDGE, and the iteration sizes
ramp 4,8,16,...,16,4 row-groups so the first matmul starts ~4us in and the
drain tail is short.  ScalarE squares X into the comb tile, VectorE casts
X and r to bf16, and a tiny tail contracts psum against
Mext = [-2*mus | 1 | mu2] to a per-core scalar.  Host sums the 8 partials.
"""

from contextlib import ExitStack

import numpy as np

import concourse.bass as bass
import concourse.tile as tile
from concourse import bacc, mybir
from concourse.bass_utils import run_bass_kernel_spmd

N, M, D = 131072, 128, 128
NCORES = 8
NSHARD = N // NCORES          # 16384 rows per core
P = 128                       # SBUF partitions
GMAX = 32                     # row-groups per iteration at steady state
# rows per iteration = P * G_i; ramp in and out for pipeline fill/drain
GSCHED = [4, 8, GMAX, GMAX, GMAX, 8, 8, 4]
assert sum(GSCHED) == NSHARD // P
W = 2 * D + 1                 # moving cols per group: [x(128) | x^2(128) | 1]

F32 = mybir.dt.float32
BF16 = mybir.dt.bfloat16
_cache: dict[str, object] = {}


def _build():
    nc = bacc.Bacc(
        "TRN2",
        target_bir_lowering=False,
        debug=False,
        num_devices=NCORES,
    )

    x_d = nc.dram_tensor("X", [NSHARD, D], F32, kind="ExternalInput").ap()
    r_d = nc.dram_tensor("r", [NSHARD, M], F32, kind="ExternalInput").ap()
    mus_d = nc.dram_tensor("mus", [M, D], F32, kind="ExternalInput").ap()
    out_d = nc.dram_tensor("out", [1, 1], F32, kind="ExternalOutput").ap()

    NGRP = sum(GSCHED)

    with tile.TileContext(nc) as tc, ExitStack() as ctx:
        singles = ctx.enter_context(tc.tile_pool(name="singles", bufs=1))
        big = ctx.enter_context(tc.tile_pool(name="big", bufs=3))
        psum_pool = ctx.enter_context(tc.tile_pool(name="psum", bufs=1, space="PSUM"))

        mus_t = singles.tile([M, D], F32)
        nc.gpsimd.dma_start(mus_t[:], mus_d[:])

        ones_col = singles.tile([P, 1], F32)
        nc.vector.memset(ones_col[:], 1.0)

        psum_acc = psum_pool.tile([M, W], F32)

        row0 = 0
        grp0 = 0
        for it, G in enumerate(GSCHED):
            rows = P * G
            # row = row0 + p*G + g  ->  AP dims [p, g, d]
            x_src = x_d[row0 : row0 + rows, :].rearrange("(p g) d -> p g d", g=G)
            r_src = r_d[row0 : row0 + rows, :].rearrange("(p g) m -> p g m", g=G)

            x_f32 = big.tile([P, G, D], F32, tag="x_f32")
            r_f32 = big.tile([P, G, M], F32, tag="r_f32")
            comb = big.tile([P, G, W], BF16, tag="comb")
            r_bf = big.tile([P, G, M], BF16, tag="r_bf")

            # Cold-start: the sync HWDGE ring serializes its first DMAs
            # (~2.5us receipt each), so the first two r-loads ride the scalar
            # HWDGE ring to overlap with the x-loads on sync.
            nc.sync.dma_start(x_f32[:], x_src)
            if it < 2:
                nc.scalar.dma_start(r_f32[:], r_src)
            else:
                nc.sync.dma_start(r_f32[:], r_src)

            nc.vector.tensor_copy(comb[:, :, 0:D], x_f32[:])
            nc.scalar.activation(
                comb[:, :, D : 2 * D], x_f32[:], mybir.ActivationFunctionType.Square
            )
            nc.vector.tensor_copy(r_bf[:], r_f32[:])
            nc.vector.memset(comb[:, :, 2 * D : 2 * D + 1], 1.0)

            for g in range(G):
                idx = grp0 + g
                nc.tensor.matmul(
                    psum_acc[:],
                    r_bf[:, g, :],
                    comb[:, g, :],
                    start=(idx == 0),
                    stop=(idx == NGRP - 1),
                )
            row0 += rows
            grp0 += G

            if it == 2:
                # Mext = [-2*mus | 1 | mu2]; built mid-kernel so it neither
                # delays the first DMAs nor sits on the critical tail path.
                mext = singles.tile([M, W], F32)
                nc.scalar.mul(mext[:, 0:D], mus_t[:], -2.0)
                nc.vector.memset(mext[:, D : 2 * D], 1.0)
                mus_sq = singles.tile([M, D], F32)
                nc.vector.tensor_mul(mus_sq[:], mus_t[:], mus_t[:])
                nc.vector.reduce_sum(
                    mext[:, 2 * D : 2 * D + 1], mus_sq[:], axis=mybir.AxisListType.X
                )

        # Tail: scalar partial = sum(psum_acc * Mext)
        prod = singles.tile([M, W], F32)
        nc.vector.tensor_mul(prod[:], psum_acc[:], mext[:])
        u = singles.tile([M, 1], F32)
        nc.vector.reduce_sum(u[:], prod[:], axis=mybir.AxisListType.X)
        psum_s = psum_pool.tile([1, 1], F32)
        nc.tensor.matmul(psum_s[:], u[:], ones_col[:], start=True, stop=True)
        res = singles.tile([1, 1], F32)
        nc.vector.tensor_copy(res[:], psum_s[:])
        nc.sync.dma_start(out_d[:], res[:])

    nc.compile()
    return nc


def _get_nc():
    if "nc" not in _cache:
        _cache["nc"] = _build()
    return _cache["nc"]


def _run(X, r, mus, **spmd_kwargs):
    X = np.ascontiguousarray(np.asarray(X, dtype=np.float32))
    r = np.ascontiguousarray(np.asarray(r, dtype=np.float32))
    mus = np.ascontiguousarray(np.asarray(mus, dtype=np.float32))
    assert X.shape == (N, D) and r.shape == (N, M) and mus.shape == (M, D)

    nc = _get_nc()
    in_maps = [
        {
            "X": X[c * NSHARD : (c + 1) * NSHARD],
            "r": r[c * NSHARD : (c + 1) * NSHARD],
            "mus": mus,
        }
        for c in range(NCORES)
    ]
    return run_bass_kernel_spmd(nc, in_maps, core_ids=list(range(NCORES)), **spmd_kwargs)


def kernel(X, r, mus):
    out = _run(X, r, mus)
    total = sum(float(out.results[c]["out"][0, 0]) for c in range(NCORES))
    return np.float32(total / N)


# revision 13
# speedup vs baseline: 1.1273x; 1.0195x over previous
"""IsometricLoss on 8 Trainium2 NeuronCores (data-parallel over N).

loss = sum(r * max(||x||^2 + ||mu||^2 - 2 x.mu, 0)) / N

For random-normal X/mus the squared distances are ~2*D >> fp32 noise, so the
max(.,0) clamp never binds and the loss decomposes exactly:

  loss*N = sum_{m,d} S1[m,d] * (-2 mus[m,d])     (cross term,  S1 = r^T X)
         + sum_{m,d} S2[m,d]                     (||x||^2 term, S2 = r^T X.^2)
         + sum_m    rc[m] * mu2[m]               (||mu||^2 term, rc = r^T 1)

Each core streams its N/8 shard of X and r exactly once (memory-bound).
Rows are packed 16-per-partition so each DMA descriptor moves 8 KiB.
Per 128-row group a single bf16 PE matmul accumulates

    psum[128, 257] += r_grp^T @ [x_bf16 | x.^2_bf16 | ones]

into one PSUM bank (fp32 accumulate).  bf16 inputs are safe: the loss is
linear in r and the round-to-nearest input errors (~2^-9 relative, zero
mean) cancel statistically over the 16M-product sums (~1e-6 net).

Pipelining: X rides the Sync HWDGE ring while r rides the Scalar HWDGE ring
(parallel first loads), mus goes via GpSimd SWDGE, and the iteration sizes
ramp 4,8,16,...,16,4 row-groups so the first matmul starts ~4us in and the
drain tail is short.  ScalarE squares X into the comb tile, VectorE casts
X and r to bf16, and a tiny tail contracts psum against
Mext = [-2*mus | 1 | mu2] to a per-core scalar.  Host sums the 8 partials.
"""

from contextlib import ExitStack

import numpy as np

import concourse.bass as bass
import concourse.tile as tile
from concourse import bacc, mybir
from concourse.bass_utils import run_bass_kernel_spmd

N, M, D = 131072, 128, 128
NCORES = 8
NSHARD = N // NCORES          # 16384 rows per core
P = 128                       # SBUF partitions
GMAX = 16                     # row-groups per iteration at steady state
# rows per iteration = P * G_i; ramp in and out for pipeline fill/drain
GSCHED = [4, 8] + [GMAX] * 6 + [8, 8, 4]
assert sum(GSCHED) == NSHARD // P
W = 2 * D + 1                 # moving cols per group: [x(128) | x^2(128) | 1]

F32 = mybir.dt.float32
BF16 = mybir.dt.bfloat16
_cache: dict[str, object] = {}


def _build():
    nc = bacc.Bacc(
        "TRN2",
        target_bir_lowering=False,
        debug=False,
        num_devices=NCORES,
    )

    x_d = nc.dram_tensor("X", [NSHARD, D], F32, kind="ExternalInput").ap()
    r_d = nc.dram_tensor("r", [NSHARD, M], F32, kind="ExternalInput").ap()
    mus_d = nc.dram_tensor("mus", [M, D], F32, kind="ExternalInput").ap()
    out_d = nc.dram_tensor("out", [1, 1], F32, kind="ExternalOutput").ap()

    NGRP = sum(GSCHED)

    with tile.TileContext(nc) as tc, ExitStack() as ctx:
        singles = ctx.enter_context(tc.tile_pool(name="singles", bufs=1))
        big = ctx.enter_context(tc.tile_pool(name="big", bufs=4))
        psum_pool = ctx.enter_context(tc.tile_pool(name="psum", bufs=1, space="PSUM"))

        mus_t = singles.tile([M, D], F32)
        nc.gpsimd.dma_start(mus_t[:], mus_d[:])

        ones_col = singles.tile([P, 1], F32)
        nc.vector.memset(ones_col[:], 1.0)

        psum_acc = psum_pool.tile([M, W], F32)

        row0 = 0
        grp0 = 0
        for it, G in enumerate(GSCHED):
            rows = P * G
            # row = row0 + p*G + g  ->  AP dims [p, g, d]
            x_src = x_d[row0 : row0 + rows, :].rearrange("(p g) d -> p g d", g=G)
            r_src = r_d[row0 : row0 + rows, :].rearrange("(p g) m -> p g m", g=G)

            x_f32 = big.tile([P, G, D], F32, tag="x_f32")
            r_f32 = big.tile([P, G, M], F32, tag="r_f32")
            comb = big.tile([P, G, W], BF16, tag="comb")
            r_bf = big.tile([P, G, M], BF16, tag="r_bf")

            # Cold-start: the sync HWDGE ring serializes its first DMAs
            # (~2.5us receipt each), so the first two r-loads ride the scalar
            # HWDGE ring to overlap with the x-loads on sync.
            nc.sync.dma_start(x_f32[:], x_src)
            if it < 2:
                nc.scalar.dma_start(r_f32[:], r_src)
            else:
                nc.sync.dma_start(r_f32[:], r_src)

            # Split each iteration in half so the first matmuls only wait on
            # half of the cast/square latency.
            halves = [(0, G // 2), (G // 2, G)] if G > 1 else [(0, G)]
            for lo, hi in halves:
                nc.vector.tensor_copy(comb[:, lo:hi, 0:D], x_f32[:, lo:hi, :])
                nc.scalar.activation(
                    comb[:, lo:hi, D : 2 * D],
                    x_f32[:, lo:hi, :],
                    mybir.ActivationFunctionType.Square,
                )
                nc.vector.tensor_copy(r_bf[:, lo:hi, :], r_f32[:, lo:hi, :])
                nc.vector.memset(comb[:, lo:hi, 2 * D : 2 * D + 1], 1.0)

                for g in range(lo, hi):
                    idx = grp0 + g
                    nc.tensor.matmul(
                        psum_acc[:],
                        r_bf[:, g, :],
                        comb[:, g, :],
                        start=(idx == 0),
                        stop=(idx == NGRP - 1),
                    )
            row0 += rows
            grp0 += G

            if it == 2:
                # Mext = [-2*mus | 1 | mu2]; built mid-kernel so it neither
                # delays the first DMAs nor sits on the critical tail path.
                mext = singles.tile([M, W], F32)
                nc.scalar.mul(mext[:, 0:D], mus_t[:], -2.0)
                nc.vector.memset(mext[:, D : 2 * D], 1.0)
                mus_sq = singles.tile([M, D], F32)
                nc.vector.tensor_mul(mus_sq[:], mus_t[:], mus_t[:])
                nc.vector.reduce_sum(
                    mext[:, 2 * D : 2 * D + 1], mus_sq[:], axis=mybir.AxisListType.X
                )

        # Tail: scalar partial = sum(psum_acc * Mext)
        prod = singles.tile([M, W], F32)
        nc.vector.tensor_mul(prod[:], psum_acc[:], mext[:])
        u = singles.tile([M, 1], F32)
        nc.vector.reduce_sum(u[:], prod[:], axis=mybir.AxisListType.X)
        psum_s = psum_pool.tile([1, 1], F32)
        nc.tensor.matmul(psum_s[:], u[:], ones_col[:], start=True, stop=True)
        res = singles.tile([1, 1], F32)
        nc.vector.tensor_copy(res[:], psum_s[:])
        nc.sync.dma_start(out_d[:], res[:])

    nc.compile()
    return nc


def _get_nc():
    if "nc" not in _cache:
        _cache["nc"] = _build()
    return _cache["nc"]


def _run(X, r, mus, **spmd_kwargs):
    X = np.ascontiguousarray(np.asarray(X, dtype=np.float32))
    r = np.ascontiguousarray(np.asarray(r, dtype=np.float32))
    mus = np.ascontiguousarray(np.asarray(mus, dtype=np.float32))
    assert X.shape == (N, D) and r.shape == (N, M) and mus.shape == (M, D)

    nc = _get_nc()
    in_maps = [
        {
            "X": X[c * NSHARD : (c + 1) * NSHARD],
            "r": r[c * NSHARD : (c + 1) * NSHARD],
            "mus": mus,
        }
        for c in range(NCORES)
    ]
    return run_bass_kernel_spmd(nc, in_maps, core_ids=list(range(NCORES)), **spmd_kwargs)


def kernel(X, r, mus):
    out = _run(X, r, mus)
    total = sum(float(out.results[c]["out"][0, 0]) for c in range(NCORES))
    return np.float32(total / N)
